# revision 15
# baseline (speedup 1.0000x reference)
"""Bass/Tile TRN2 kernel for nn_MultiHeadAttention_9277129359942.

B=2, T=S=2048, D=1024, H=16 heads, head_dim=64, fp32 I/O.

Sharding (8 cores): data-parallel over batch (2) x tensor-parallel over
head groups (4 heads / core, 256 out dims).  Each core computes the
attention for its 4 heads and a partial output projection; the host sums
the 4 partials per batch (row-parallel Wo) and adds bo once.

Device-side layout (transpose-free):
  - activations arrive feature-major, pre-cast:  x^T [D, T] bf16
  - weights arrive as W^T slices in bf16: wq/wk/wv [1024, 256], wo [256, 1024]
  - q,k produced transposed ([256, 2048], head dim on partitions); v in
    natural [S, 256] layout with a ones column per head (v_aug) so the
    attention's second matmul also produces the softmax denominator.
  - scores computed transposed (s on partitions, t free); softmax skips
    max-subtraction (scores ~ N(0,1), exp cannot overflow fp32/bf16).

Schedule: input DMAs are three whole-tensor transfers on the sync HWDGE
ring (xv, xk, xq) with weights on the scalar ring; projections run in
the shadow of the loads (v, k-m0, q-m0, k-m1, q-m1, double-buffered
4-bank PSUM); the attention loop then owns all 8 PSUM banks (2x sc
double-buffered + 2x ctx accumulators) and is Scalar-engine-bound on
exp.  A fraction of the exp tiles can be offloaded to the DVE using a
bf16 Schraudolph bit-trick (EXP_APPROX_FRAC).  Softmax normalization
uses reciprocal_approx_fast and a log2 DMA broadcast chain.  The output
projection runs at the tail with PSUM eviction alternating DVE/Scalar,
bf16 output, bo added on host.
"""

import os
import sys

import numpy as np

for _p in ("/opt/trn_rl_repo",):
    if os.path.isdir(_p) and _p not in sys.path:
        sys.path.append(_p)

import ml_dtypes

import concourse.bass as bass
import concourse.mybir as mybir
import concourse.tile as tile
from concourse import bacc
from concourse.bass_utils import run_bass_kernel_spmd

F32 = mybir.dt.float32
BF16 = mybir.dt.bfloat16
I16 = mybir.dt.int16
AF = mybir.ActivationFunctionType
ALU = mybir.AluOpType
BF16_NP = ml_dtypes.bfloat16

D = 1024          # model dim
T = 2048          # query length
S = 2048          # key length
P = 128           # partitions
KT = D // P       # 8 contraction tiles
TT = T // P       # 16 row tiles
ST = S // P       # 16 key tiles
HL = 4            # local heads per core
HD = 64           # head dim
OUTL = HL * HD    # 256 local out dims
ONES_W = 32       # denominator replica rows: reciprocal_approx_fast
                  # needs base partition 0 and partition bases must be
                  # 32-aligned; kept as narrow as alignment allows --
                  # wide replicas burn PE power (P0 downclock risk)
VW = ONES_W + HD  # v_aug width per head: [32 ones | 64 v]
N_CORES = 8

# Schraudolph bf16 exp on DVE for a fraction of the (s, i) tiles:
#   bits(exp(x*0.125)) ~= round(x * 0.125*128/ln2 + (16256 - C))
EXP_A = 0.125 * 128.0 / float(np.log(2.0))
EXP_C = 5.0
# which (s, i) tiles go to the DVE (per block); tuned for error budget
APPROX_SET = frozenset()  # Step 1: none (exact). Step 2: see make_approx_set
DEBUG_EX = False


def make_approx_set(frac_num, frac_den):
    sel = set()
    for s in range(ST):
        for i in range(2):
            if ((s * 2 + i) * frac_num) % frac_den < frac_num:
                sel.add((s, i))
    return frozenset(sel)


def build_program(approx_set=APPROX_SET):
    nc = bacc.Bacc(
        "TRN2", target_bir_lowering=False, debug=False, enable_asserts=True,
        num_devices=N_CORES,
    )

    xq_d = nc.dram_tensor("xq", [D, T], BF16, kind="ExternalInput")
    xk_d = nc.dram_tensor("xk", [D, S], BF16, kind="ExternalInput")
    xv_d = nc.dram_tensor("xv", [D, S], BF16, kind="ExternalInput")
    wq_d = nc.dram_tensor("wq", [D, OUTL], BF16, kind="ExternalInput")
    wk_d = nc.dram_tensor("wk", [D, OUTL], BF16, kind="ExternalInput")
    wv_d = nc.dram_tensor("wv", [D, OUTL], BF16, kind="ExternalInput")
    wo_d = nc.dram_tensor("wo", [OUTL, D], BF16, kind="ExternalInput")
    bq_d = nc.dram_tensor("bq", [OUTL, 1], F32, kind="ExternalInput")
    bk_d = nc.dram_tensor("bk", [OUTL, 1], F32, kind="ExternalInput")
    bv_d = nc.dram_tensor("bv_rep", [P, OUTL], F32, kind="ExternalInput")
    out_d = nc.dram_tensor("out", [T, D], BF16, kind="ExternalOutput")
    wsink_d = nc.dram_tensor("warm_sink", [1, 8], F32, kind="ExternalOutput")
    dbg_d = None
    if DEBUG_EX:
        dbg_d = nc.dram_tensor("dbg_ex", [P, 1024], BF16, kind="ExternalOutput")

    with tile.TileContext(nc) as tc:
        _build(nc, tc, xq_d, xk_d, xv_d, wq_d, wk_d, wv_d, wo_d,
               bq_d, bk_d, bv_d, out_d, wsink_d, approx_set, dbg_d)
    nc.compile()
    return nc


def _build(nc, tc, xq_d, xk_d, xv_d, wq_d, wk_d, wv_d, wo_d,
           bq_d, bk_d, bv_d, out_d, wsink_d, approx_set, dbg_d):
    from contextlib import ExitStack

    stack = ExitStack()
    with stack:
        consts = stack.enter_context(tc.tile_pool(name="consts", bufs=1))
        wpool = stack.enter_context(tc.tile_pool(name="wpool", bufs=1))
        acts = stack.enter_context(tc.tile_pool(name="acts", bufs=1))

        # ---- weights + biases on the scalar (ACT) HWDGE ring ----------
        wv_sb = wpool.tile([P, KT * OUTL], BF16, name="wv", tag="wv")
        wk_sb = wpool.tile([P, KT * OUTL], BF16, name="wk", tag="wk")
        wq_sb = wpool.tile([P, KT * OUTL], BF16, name="wq", tag="wq")
        wo_sb = wpool.tile([P, 2 * D], BF16, name="wo", tag="wo")
        bv_sb = consts.tile([P, OUTL], F32, name="bv", tag="bv")
        bq_sb = consts.tile([P, 2], F32, name="bq", tag="bq")
        bk_sb = consts.tile([P, 2], F32, name="bk", tag="bk")

        def load_tiled(eng, sb, d_, inner):
            eng.dma_start(sb[:].rearrange("p (k o) -> p k o", o=inner),
                          d_.rearrange("(k p) o -> p k o", p=P))

        load_tiled(nc.scalar, wv_sb, wv_d, OUTL)
        nc.scalar.dma_start(bv_sb[:], bv_d[:, :])
        load_tiled(nc.scalar, wk_sb, wk_d, OUTL)
        nc.scalar.dma_start(bk_sb[:], bk_d.rearrange("(m p) o -> p (m o)", p=P))
        load_tiled(nc.scalar, wq_sb, wq_d, OUTL)
        nc.scalar.dma_start(bq_sb[:], bq_d.rearrange("(m p) o -> p (m o)", p=P))
        load_tiled(nc.scalar, wo_sb, wo_d, D)

        # ---- x tensors: whole-tensor DMAs on the sync (SP) ring --------
        xpool = stack.enter_context(tc.tile_pool(name="xpool", bufs=1))
        xv_sb = xpool.tile([P, KT * S], BF16, name="xv", tag="xv")
        xk_sb = xpool.tile([P, KT * S], BF16, name="xk", tag="xk")
        xq_sb = xpool.tile([P, KT * T], BF16, name="xq", tag="xq")
        load_tiled(nc.sync, xv_sb, xv_d, S)
        load_tiled(nc.sync, xk_sb, xk_d, S)
        load_tiled(nc.sync, xq_sb, xq_d, T)

        # persistent activations
        qT = [acts.tile([P, T], BF16, name=f"qT{m}", tag=f"qT{m}")
              for m in range(2)]
        kT = [acts.tile([P, S], BF16, name=f"kT{m}", tag=f"kT{m}")
              for m in range(2)]
        v_aug = acts.tile([P, ST * HL * VW], BF16, name="vaug", tag="vaug")
        ctxT = [[acts.tile([P, 1024], BF16, name=f"ctxT{p}{th}",
                           tag=f"ctxT{p}{th}") for th in range(2)]
                for p in range(2)]
        nc.vector.memset(v_aug[:], 1.0)  # ones columns survive the v writes

        # ---- warmup + v projection ------------------------------------
        with tc.tile_pool(name="vpsum", bufs=2, space="PSUM") as vpsum:
            # HAM warmup: dense matmul burst on (not-yet-written) kT/qT
            # tiles to un-throttle the PE clock; sunk to an output so it
            # is not dead-code-eliminated.
            warm_ps = None
            for grp in range(2):
                warm_ps = vpsum.tile([P, 512], F32, name="warm", tag="warm")
                for w in range(8):
                    nc.tensor.matmul(warm_ps[:], kT[0][0:HD, 0:P],
                                     qT[0][0:HD, 0:512],
                                     start=(w == 0), stop=(w == 7))
            wsnk = consts.tile([1, 8], F32, name="wsnk", tag="wsnk")
            nc.vector.tensor_copy(wsnk[:], warm_ps[0:1, 0:8])
            nc.sync.dma_start(wsink_d[:, :], wsnk[:])

            bv3 = bv_sb[:].rearrange("p (h x) -> p h x", x=HD)
            for s in range(ST):
                ps = vpsum.tile([P, OUTL], F32, name="pv", tag="pv")
                for k in range(KT):
                    nc.tensor.matmul(
                        ps[:], xv_sb[:, k * S + s * P: k * S + (s + 1) * P],
                        wv_sb[:, k * OUTL:(k + 1) * OUTL],
                        start=(k == 0), stop=(k == KT - 1))
                dst = v_aug[:, s * HL * VW:(s + 1) * HL * VW]
                dst = dst.rearrange("p (h x) -> p h x", x=VW)[:, :, ONES_W:VW]
                nc.vector.tensor_tensor(
                    out=dst, in0=ps[:].rearrange("p (h x) -> p h x", x=HD),
                    in1=bv3, op=ALU.add)

        # ---- q/k projections: 5 phases, double-buffered 4-bank PSUM ----
        # out[m] = (W[:, m*128:(m+1)*128]).T @ x   -> [128, 2048]
        # c-outer / k-inner so each 512-col chunk evicts early.
        with tc.tile_pool(name="qkpsum", bufs=2, space="PSUM") as qkpsum:
            for w_sb, x_sb, b_sb, o_sb, m in (
                    (wk_sb, xk_sb, bk_sb, kT, 0),
                    (wk_sb, xk_sb, bk_sb, kT, 1),
                    (wq_sb, xq_sb, bq_sb, qT, 0),
                    (wq_sb, xq_sb, bq_sb, qT, 1)):
                ps = qkpsum.tile([P, T], F32, name=f"pqk{m}", tag="pqk")
                for c in range(4):
                    cs = slice(c * 512, (c + 1) * 512)
                    for k in range(KT):
                        nc.tensor.matmul(
                            ps[:, cs],
                            w_sb[:, k * OUTL + m * P: k * OUTL + (m + 1) * P],
                            x_sb[:, k * T + c * 512: k * T + (c + 1) * 512],
                            start=(k == 0), stop=(k == KT - 1))
                    nc.vector.tensor_scalar_add(o_sb[m][:, cs], ps[:, cs],
                                                b_sb[:, m:m + 1])

        # ---- attention ------------------------------------------------
        with tc.tile_pool(name="spsum", bufs=1, space="PSUM") as spsum, \
             tc.tile_pool(name="cpsum", bufs=1, space="PSUM") as cpsum, \
             tc.tile_pool(name="epool", bufs=2) as epool, \
             tc.tile_pool(name="npool", bufs=2) as npool, \
             tc.tile_pool(name="opool", bufs=4) as opool:

            def emit_norm(p, th, ctx_ps):
                """ctx psum rows 0:32 hold denominator replicas, rows
                32:96 the unnormalized ctx.  Evict to SBUF first (frees
                the psum banks for the next block ~2us after the last
                ctx matmul), then fast reciprocal at base partition 0,
                a 2-hop DMA fill, two naturally-aligned 32-row
                multiplies, and a partition-shift DMA into ctxT."""
                ce = ONES_W + HD
                stgs = []
                for i in range(2):
                    stg = npool.tile([P, 1024], F32, name=f"stg{i}",
                                     tag=f"stg{i}")
                    nc.vector.tensor_copy(stg[0:ce, :], ctx_ps[i][0:ce, :])
                    stgs.append(stg)
                for i in range(2):
                    stg = stgs[i]
                    rb = npool.tile([P, 1024], F32, name=f"rb{i}",
                                    tag=f"rb{i}")
                    nc.vector.reciprocal_approx_fast(
                        rb[0:ONES_W, :], stg[0:ONES_W, :])
                    nc.sync.dma_start(rb[ONES_W:2 * ONES_W, :],
                                      rb[0:ONES_W, :])
                    nc.sync.dma_start(rb[2 * ONES_W:ce, :], rb[0:ONES_W, :])
                    ostg = npool.tile([P, 1024], BF16, name=f"ostg{i}",
                                      tag=f"ostg{i}")
                    # partition accesses must be naturally aligned: the
                    # 64-row multiply at base 32 splits into two 32-row ops
                    for r in (slice(ONES_W, 2 * ONES_W),
                              slice(2 * ONES_W, ce)):
                        nc.vector.tensor_tensor(
                            out=ostg[r, :], in0=stg[r, :],
                            in1=rb[r, :], op=ALU.mult)
                    nc.sync.dma_start(
                        ctxT[p][th][i * HD:(i + 1) * HD, :],
                        ostg[ONES_W:ce, :])

            for p, th in ((0, 0), (1, 0), (0, 1), (1, 1)):
                t0 = th * 1024
                ctx_ps = [cpsum.tile([VW, 1024], F32, name=f"ctx{i}",
                                     tag=f"ctx{i}") for i in range(2)]
                for s in range(ST):
                    sc = [spsum.tile([P, 1024], F32, name=f"sc{i}",
                                     tag=f"sc{i}") for i in range(2)]
                    ss = slice(s * P, (s + 1) * P)
                    # two heads' matmuls interleave into disjoint PE row
                    # groups (K=64 each) and run concurrently
                    for c in range(2):
                        cs_o = slice(c * 512, (c + 1) * 512)
                        cs_q = slice(t0 + c * 512, t0 + (c + 1) * 512)
                        for i in range(2):
                            hp = slice(i * HD, (i + 1) * HD)
                            nc.tensor.matmul(
                                sc[i][:, cs_o], kT[p][hp, ss],
                                qT[p][hp, cs_q], start=True, stop=True)
                    ex = []
                    for i in range(2):
                        e = epool.tile([P, 1024], BF16, name=f"ex{i}",
                                       tag=f"ex{i}")
                        if (s, i) in approx_set:
                            nc.vector.tensor_scalar(
                                out=e[:].bitcast(I16), in0=sc[i][:],
                                scalar1=EXP_A, scalar2=16256.0 - EXP_C,
                                op0=ALU.mult, op1=ALU.add)
                        else:
                            nc.scalar.activation(e[:], sc[i][:], AF.Exp,
                                                 scale=0.125)
                        if dbg_d is not None and (p, th, s, i) == (0, 0, 0, 0):
                            nc.sync.dma_start(dbg_d[:, :], e[:])
                        ex.append(e)
                    for i in range(2):
                        h = 2 * p + i
                        vs = slice(s * HL * VW + h * VW,
                                   s * HL * VW + h * VW + VW)
                        for c in range(2):
                            cs_o = slice(c * 512, (c + 1) * 512)
                            nc.tensor.matmul(
                                ctx_ps[i][:, cs_o], v_aug[:, vs],
                                ex[i][:, cs_o],
                                start=(s == 0), stop=(s == ST - 1))
                emit_norm(p, th, ctx_ps)

            # ---- output projection (tail) -----------------------------
            # th=0 tiles evict via the (now idle) Scalar engine so the
            # DVE can finish the last block's normalize concurrently;
            # th=1 tiles alternate DVE/Scalar.
            for t in range(TT):
                th, tl = divmod(t, TT // 2)
                ts = slice(tl * P, (tl + 1) * P)
                ps = spsum.tile([P, D], F32, name="po", tag=f"sc{t % 2}")
                for p2 in range(2):
                    for n in range(2):
                        ns = slice(n * 512, (n + 1) * 512)
                        nc.tensor.matmul(
                            ps[:, ns], ctxT[p2][th][:, ts],
                            wo_sb[:, p2 * D + n * 512: p2 * D + (n + 1) * 512],
                            start=(p2 == 0), stop=(p2 == 1))
                ost = opool.tile([P, D], BF16, name="ost", tag="ost")
                if th == 0 or t % 2 == 1:
                    nc.scalar.activation(ost[:], ps[:], AF.Copy)
                else:
                    nc.vector.tensor_copy(ost[:], ps[:])
                rings = ((nc.scalar, nc.gpsimd) if th == 0
                         else (nc.sync, nc.scalar, nc.gpsimd))
                rings[t % len(rings)].dma_start(
                    out_d[t * P:(t + 1) * P, :], ost[:])


def make_in_maps(query, key, value, Wq, bq, Wk, bk, Wv, bv, Wo, bo):
    """Shard the full inputs into the 8 per-core input dicts."""
    query, key, value, Wq, bq, Wk, bk, Wv, bv, Wo, bo = [
        np.asarray(a, dtype=np.float32)
        for a in (query, key, value, Wq, bq, Wk, bk, Wv, bv, Wo, bo)]

    def bf(a):
        return np.ascontiguousarray(a).astype(BF16_NP)

    in_maps = []
    for c in range(N_CORES):
        b, g = divmod(c, 4)
        sl = slice(g * OUTL, (g + 1) * OUTL)
        in_maps.append({
            "xq": bf(query[b].T),
            "xk": bf(key[b].T),
            "xv": bf(value[b].T),
            "wq": bf(Wq[sl, :].T),
            "wk": bf(Wk[sl, :].T),
            "wv": bf(Wv[sl, :].T),
            "wo": bf(Wo[:, sl].T),
            "bq": np.ascontiguousarray(bq[sl].reshape(OUTL, 1)),
            "bk": np.ascontiguousarray(bk[sl].reshape(OUTL, 1)),
            "bv_rep": np.ascontiguousarray(
                np.broadcast_to(bv[sl], (P, OUTL))),
        })
    return in_maps


_NC_CACHE = None


def _get_nc():
    global _NC_CACHE
    if _NC_CACHE is None:
        _NC_CACHE = build_program()
    return _NC_CACHE


def gather_out(results, bo):
    out = np.empty((2, T, D), dtype=np.float32)
    bo = np.asarray(bo, dtype=np.float32)
    for b in range(2):
        acc = results[4 * b]["out"].astype(np.float32)
        for g in range(1, 4):
            acc = acc + results[4 * b + g]["out"].astype(np.float32)
        out[b] = acc + bo
    return out


def kernel(query, key, value, Wq, bq, Wk, bk, Wv, bv, Wo, bo):
    nc = _get_nc()
    in_maps = make_in_maps(query, key, value, Wq, bq, Wk, bk, Wv, bv, Wo, bo)
    res = run_bass_kernel_spmd(nc, in_maps, list(range(N_CORES)))
    return gather_out(res.results, bo)


# revision 16
# speedup vs baseline: 1.4581x; 1.4581x over previous
"""Bass/Tile TRN2 kernel for nn_MultiHeadAttention_9277129359942.

B=2, T=S=2048, D=1024, H=16 heads, head_dim=64, fp32 I/O.

Sharding (8 cores): data-parallel over batch (2) x tensor-parallel over
head groups (4 heads / core, 256 out dims).  Each core computes the
attention for its 4 heads and a partial output projection; the host sums
the 4 partials per batch (row-parallel Wo) and adds bo once.

Device-side layout (transpose-free):
  - activations arrive feature-major, pre-cast:  x^T [D, T] bf16
  - weights arrive as W^T slices in bf16: wq/wk/wv [1024, 256], wo [256, 1024]
  - q,k produced transposed ([256, 2048], head dim on partitions); v in
    natural [S, 256] layout with a ones column per head (v_aug) so the
    attention's second matmul also produces the softmax denominator.
  - scores computed transposed (s on partitions, t free); softmax skips
    max-subtraction (scores ~ N(0,1), exp cannot overflow fp32/bf16).

Schedule: input DMAs are three whole-tensor transfers on the sync HWDGE
ring (xv, xk, xq) with weights on the scalar ring; projections run in
the shadow of the loads (v, k-m0, q-m0, k-m1, q-m1, double-buffered
4-bank PSUM); the attention loop then owns all 8 PSUM banks (2x sc
double-buffered + 2x ctx accumulators) and is Scalar-engine-bound on
exp.  A fraction of the exp tiles can be offloaded to the DVE using a
bf16 Schraudolph bit-trick (EXP_APPROX_FRAC).  Softmax normalization
uses reciprocal_approx_fast and a log2 DMA broadcast chain.  The output
projection runs at the tail with PSUM eviction alternating DVE/Scalar,
bf16 output, bo added on host.
"""

import os
import sys

import numpy as np

for _p in ("/opt/trn_rl_repo",):
    if os.path.isdir(_p) and _p not in sys.path:
        sys.path.append(_p)

import ml_dtypes

import concourse.bass as bass
import concourse.mybir as mybir
import concourse.tile as tile
from concourse import bacc
from concourse.bass_utils import run_bass_kernel_spmd

F32 = mybir.dt.float32
BF16 = mybir.dt.bfloat16
I16 = mybir.dt.int16
AF = mybir.ActivationFunctionType
ALU = mybir.AluOpType
BF16_NP = ml_dtypes.bfloat16

D = 1024          # model dim
T = 2048          # query length
S = 2048          # key length
P = 128           # partitions
KT = D // P       # 8 contraction tiles
TT = T // P       # 16 row tiles
ST = S // P       # 16 key tiles
HL = 4            # local heads per core
HD = 64           # head dim
OUTL = HL * HD    # 256 local out dims
ONES_W = 32       # denominator replica rows: reciprocal_approx_fast
                  # needs base partition 0 and partition bases must be
                  # 32-aligned; kept as narrow as alignment allows --
                  # wide replicas burn PE power (P0 downclock risk)
VW = ONES_W + HD  # v_aug width per head: [32 ones | 64 v]
N_CORES = 8

# Schraudolph bf16 exp on DVE for a fraction of the (s, i) tiles:
#   bits(exp(x*0.125)) ~= round(x * 0.125*128/ln2 + (16256 - C))
EXP_A = 0.125 * 128.0 / float(np.log(2.0))
EXP_C = 5.0
# which (s, i) tiles go to the DVE (per block); tuned for error budget
APPROX_SET = frozenset()  # Step 1: none (exact). Step 2: see make_approx_set
DEBUG_EX = False


def make_approx_set(frac_num, frac_den):
    sel = set()
    for s in range(ST):
        for i in range(2):
            if ((s * 2 + i) * frac_num) % frac_den < frac_num:
                sel.add((s, i))
    return frozenset(sel)


def build_program(approx_set=APPROX_SET):
    nc = bacc.Bacc(
        "TRN2", target_bir_lowering=False, debug=False, enable_asserts=True,
        num_devices=N_CORES,
    )

    xq_d = nc.dram_tensor("xq", [D, T], BF16, kind="ExternalInput")
    xk_d = nc.dram_tensor("xk", [D, S], BF16, kind="ExternalInput")
    xv_d = nc.dram_tensor("xv", [D, S], BF16, kind="ExternalInput")
    wq_d = nc.dram_tensor("wq", [D, OUTL], BF16, kind="ExternalInput")
    wk_d = nc.dram_tensor("wk", [D, OUTL], BF16, kind="ExternalInput")
    wv_d = nc.dram_tensor("wv", [D, OUTL], BF16, kind="ExternalInput")
    wo_d = nc.dram_tensor("wo", [OUTL, D], BF16, kind="ExternalInput")
    bq_d = nc.dram_tensor("bq", [OUTL, 1], F32, kind="ExternalInput")
    bk_d = nc.dram_tensor("bk", [OUTL, 1], F32, kind="ExternalInput")
    bv_d = nc.dram_tensor("bv_rep", [P, OUTL], F32, kind="ExternalInput")
    out_d = nc.dram_tensor("out", [T, D], BF16, kind="ExternalOutput")
    wsink_d = nc.dram_tensor("warm_sink", [1, 8], F32, kind="ExternalOutput")
    dbg_d = None
    if DEBUG_EX:
        dbg_d = nc.dram_tensor("dbg_ex", [P, 1024], BF16, kind="ExternalOutput")

    with tile.TileContext(nc) as tc:
        _build(nc, tc, xq_d, xk_d, xv_d, wq_d, wk_d, wv_d, wo_d,
               bq_d, bk_d, bv_d, out_d, wsink_d, approx_set, dbg_d)
    nc.compile()
    return nc


def _build(nc, tc, xq_d, xk_d, xv_d, wq_d, wk_d, wv_d, wo_d,
           bq_d, bk_d, bv_d, out_d, wsink_d, approx_set, dbg_d):
    from contextlib import ExitStack

    stack = ExitStack()
    with stack:
        consts = stack.enter_context(tc.tile_pool(name="consts", bufs=1))
        wpool = stack.enter_context(tc.tile_pool(name="wpool", bufs=1))
        acts = stack.enter_context(tc.tile_pool(name="acts", bufs=1))

        # ---- weights + biases on the scalar (ACT) HWDGE ring ----------
        wv_sb = wpool.tile([P, KT * OUTL], BF16, name="wv", tag="wv")
        wk_sb = wpool.tile([P, KT * OUTL], BF16, name="wk", tag="wk")
        wq_sb = wpool.tile([P, KT * OUTL], BF16, name="wq", tag="wq")
        wo_sb = wpool.tile([P, 2 * D], BF16, name="wo", tag="wo")
        bv_sb = consts.tile([P, OUTL], F32, name="bv", tag="bv")
        bq_sb = consts.tile([P, 2], F32, name="bq", tag="bq")
        bk_sb = consts.tile([P, 2], F32, name="bk", tag="bk")

        def load_tiled(eng, sb, d_, inner):
            eng.dma_start(sb[:].rearrange("p (k o) -> p k o", o=inner),
                          d_.rearrange("(k p) o -> p k o", p=P))

        load_tiled(nc.scalar, wv_sb, wv_d, OUTL)
        nc.scalar.dma_start(bv_sb[:], bv_d[:, :])
        load_tiled(nc.scalar, wk_sb, wk_d, OUTL)
        nc.scalar.dma_start(bk_sb[:], bk_d.rearrange("(m p) o -> p (m o)", p=P))
        load_tiled(nc.scalar, wq_sb, wq_d, OUTL)
        nc.scalar.dma_start(bq_sb[:], bq_d.rearrange("(m p) o -> p (m o)", p=P))
        load_tiled(nc.scalar, wo_sb, wo_d, D)

        # ---- x tensors: whole-tensor DMAs on the sync (SP) ring --------
        xpool = stack.enter_context(tc.tile_pool(name="xpool", bufs=1))
        xv_sb = xpool.tile([P, KT * S], BF16, name="xv", tag="xv")
        xk_sb = xpool.tile([P, KT * S], BF16, name="xk", tag="xk")
        xq_sb = xpool.tile([P, KT * T], BF16, name="xq", tag="xq")
        load_tiled(nc.sync, xv_sb, xv_d, S)
        load_tiled(nc.sync, xk_sb, xk_d, S)
        load_tiled(nc.sync, xq_sb, xq_d, T)

        # persistent activations
        qT = [acts.tile([P, T], BF16, name=f"qT{m}", tag=f"qT{m}")
              for m in range(2)]
        kT = [acts.tile([P, S], BF16, name=f"kT{m}", tag=f"kT{m}")
              for m in range(2)]
        v_aug = acts.tile([P, ST * HL * VW], BF16, name="vaug", tag="vaug")
        ctxT = [[acts.tile([P, 1024], BF16, name=f"ctxT{p}{th}",
                           tag=f"ctxT{p}{th}") for th in range(2)]
                for p in range(2)]
        nc.vector.memset(v_aug[:], 1.0)  # ones columns survive the v writes

        # ---- warmup + v projection ------------------------------------
        with tc.tile_pool(name="vpsum", bufs=2, space="PSUM") as vpsum:
            # HAM warmup: dense matmul burst on (not-yet-written) kT/qT
            # tiles to un-throttle the PE clock; sunk to an output so it
            # is not dead-code-eliminated.
            warm_ps = None
            for grp in range(2):
                warm_ps = vpsum.tile([P, 512], F32, name="warm", tag="warm")
                for w in range(8):
                    nc.tensor.matmul(warm_ps[:], kT[0][0:HD, 0:P],
                                     qT[0][0:HD, 0:512],
                                     start=(w == 0), stop=(w == 7))
            wsnk = consts.tile([1, 8], F32, name="wsnk", tag="wsnk")
            nc.vector.tensor_copy(wsnk[:], warm_ps[0:1, 0:8])
            nc.sync.dma_start(wsink_d[:, :], wsnk[:])

            bv3 = bv_sb[:].rearrange("p (h x) -> p h x", x=HD)
            for s in range(ST):
                ps = vpsum.tile([P, OUTL], F32, name="pv", tag="pv")
                for k in range(KT):
                    nc.tensor.matmul(
                        ps[:], xv_sb[:, k * S + s * P: k * S + (s + 1) * P],
                        wv_sb[:, k * OUTL:(k + 1) * OUTL],
                        start=(k == 0), stop=(k == KT - 1))
                dst = v_aug[:, s * HL * VW:(s + 1) * HL * VW]
                dst = dst.rearrange("p (h x) -> p h x", x=VW)[:, :, ONES_W:VW]
                nc.vector.tensor_tensor(
                    out=dst, in0=ps[:].rearrange("p (h x) -> p h x", x=HD),
                    in1=bv3, op=ALU.add)

        # ---- q/k projections: 5 phases, double-buffered 4-bank PSUM ----
        # out[m] = (W[:, m*128:(m+1)*128]).T @ x   -> [128, 2048]
        # c-outer / k-inner so each 512-col chunk evicts early.
        with tc.tile_pool(name="qkpsum", bufs=2, space="PSUM") as qkpsum:
            for w_sb, x_sb, b_sb, o_sb, m in (
                    (wk_sb, xk_sb, bk_sb, kT, 0),
                    (wk_sb, xk_sb, bk_sb, kT, 1),
                    (wq_sb, xq_sb, bq_sb, qT, 0),
                    (wq_sb, xq_sb, bq_sb, qT, 1)):
                ps = qkpsum.tile([P, T], F32, name=f"pqk{m}", tag="pqk")
                for c in range(4):
                    cs = slice(c * 512, (c + 1) * 512)
                    for k in range(KT):
                        nc.tensor.matmul(
                            ps[:, cs],
                            w_sb[:, k * OUTL + m * P: k * OUTL + (m + 1) * P],
                            x_sb[:, k * T + c * 512: k * T + (c + 1) * 512],
                            start=(k == 0), stop=(k == KT - 1))
                    nc.vector.tensor_scalar_add(o_sb[m][:, cs], ps[:, cs],
                                                b_sb[:, m:m + 1])

        # ---- attention ------------------------------------------------
        with tc.tile_pool(name="spsum", bufs=1, space="PSUM") as spsum, \
             tc.tile_pool(name="cpsum", bufs=1, space="PSUM") as cpsum, \
             tc.tile_pool(name="epool", bufs=2) as epool, \
             tc.tile_pool(name="npool", bufs=2) as npool, \
             tc.tile_pool(name="opool", bufs=4) as opool:

            def emit_norm(p, th, ctx_ps):
                """ctx psum rows 0:32 hold denominator replicas, rows
                32:96 the unnormalized ctx.  Evict to SBUF first (frees
                the psum banks for the next block ~2us after the last
                ctx matmul), then fast reciprocal at base partition 0,
                a 2-hop DMA fill, two naturally-aligned 32-row
                multiplies, and a partition-shift DMA into ctxT."""
                ce = ONES_W + HD
                stgs = []
                for i in range(2):
                    stg = npool.tile([P, 1024], F32, name=f"stg{i}",
                                     tag=f"stg{i}")
                    nc.vector.tensor_copy(stg[0:ce, :], ctx_ps[i][0:ce, :])
                    stgs.append(stg)
                for i in range(2):
                    stg = stgs[i]
                    rb = npool.tile([P, 1024], F32, name=f"rb{i}",
                                    tag=f"rb{i}")
                    nc.vector.reciprocal_approx_fast(
                        rb[0:ONES_W, :], stg[0:ONES_W, :])
                    nc.sync.dma_start(rb[ONES_W:2 * ONES_W, :],
                                      rb[0:ONES_W, :])
                    nc.sync.dma_start(rb[2 * ONES_W:ce, :], rb[0:ONES_W, :])
                    ostg = npool.tile([P, 1024], BF16, name=f"ostg{i}",
                                      tag=f"ostg{i}")
                    # partition accesses must be naturally aligned: the
                    # 64-row multiply at base 32 splits into two 32-row ops
                    for r in (slice(ONES_W, 2 * ONES_W),
                              slice(2 * ONES_W, ce)):
                        nc.vector.tensor_tensor(
                            out=ostg[r, :], in0=stg[r, :],
                            in1=rb[r, :], op=ALU.mult)
                    nc.sync.dma_start(
                        ctxT[p][th][i * HD:(i + 1) * HD, :],
                        ostg[ONES_W:ce, :])

            def emit_scores(p, th, s):
                """Two heads' matmuls interleave into disjoint PE row
                groups (K=64 each) and run concurrently."""
                t0 = th * 1024
                sc = [spsum.tile([P, 1024], F32, name=f"sc{i}",
                                 tag=f"sc{i}") for i in range(2)]
                ss = slice(s * P, (s + 1) * P)
                for c in range(2):
                    cs_o = slice(c * 512, (c + 1) * 512)
                    cs_q = slice(t0 + c * 512, t0 + (c + 1) * 512)
                    for i in range(2):
                        hp = slice(i * HD, (i + 1) * HD)
                        nc.tensor.matmul(
                            sc[i][:, cs_o], kT[p][hp, ss],
                            qT[p][hp, cs_q], start=True, stop=True)
                return sc

            # software-pipelined: scores(s+1) is emitted BEFORE ctx(s) so
            # the PE queue never blocks the next exp behind ctx matmuls
            # that wait on the current exp.
            blocks = ((0, 0), (1, 0), (0, 1), (1, 1))
            sc = emit_scores(*blocks[0], 0)
            for bi, (p, th) in enumerate(blocks):
                ctx_ps = [cpsum.tile([VW, 1024], F32, name=f"ctx{i}",
                                     tag=f"ctx{i}") for i in range(2)]
                for s in range(ST):
                    ex = []
                    for i in range(2):
                        e = epool.tile([P, 1024], BF16, name=f"ex{i}",
                                       tag=f"ex{i}")
                        if (s, i) in approx_set:
                            nc.vector.tensor_scalar(
                                out=e[:].bitcast(I16), in0=sc[i][:],
                                scalar1=EXP_A, scalar2=16256.0 - EXP_C,
                                op0=ALU.mult, op1=ALU.add)
                        else:
                            nc.scalar.activation(e[:], sc[i][:], AF.Exp,
                                                 scale=0.125)
                        if dbg_d is not None and (p, th, s, i) == (0, 0, 0, 0):
                            nc.sync.dma_start(dbg_d[:, :], e[:])
                        ex.append(e)
                    if s + 1 < ST:
                        sc = emit_scores(p, th, s + 1)
                    elif bi + 1 < len(blocks):
                        sc = emit_scores(*blocks[bi + 1], 0)
                    for i in range(2):
                        h = 2 * p + i
                        vs = slice(s * HL * VW + h * VW,
                                   s * HL * VW + h * VW + VW)
                        for c in range(2):
                            cs_o = slice(c * 512, (c + 1) * 512)
                            nc.tensor.matmul(
                                ctx_ps[i][:, cs_o], v_aug[:, vs],
                                ex[i][:, cs_o],
                                start=(s == 0), stop=(s == ST - 1))
                emit_norm(p, th, ctx_ps)

            # ---- output projection (tail) -----------------------------
            # th=0 tiles evict via the (now idle) Scalar engine so the
            # DVE can finish the last block's normalize concurrently;
            # th=1 tiles alternate DVE/Scalar.
            for t in range(TT):
                th, tl = divmod(t, TT // 2)
                ts = slice(tl * P, (tl + 1) * P)
                ps = spsum.tile([P, D], F32, name="po", tag=f"sc{t % 2}")
                for p2 in range(2):
                    for n in range(2):
                        ns = slice(n * 512, (n + 1) * 512)
                        nc.tensor.matmul(
                            ps[:, ns], ctxT[p2][th][:, ts],
                            wo_sb[:, p2 * D + n * 512: p2 * D + (n + 1) * 512],
                            start=(p2 == 0), stop=(p2 == 1))
                ost = opool.tile([P, D], BF16, name="ost", tag="ost")
                if th == 0 or t % 2 == 1:
                    nc.scalar.activation(ost[:], ps[:], AF.Copy)
                else:
                    nc.vector.tensor_copy(ost[:], ps[:])
                rings = ((nc.scalar, nc.gpsimd) if th == 0
                         else (nc.sync, nc.scalar, nc.gpsimd))
                rings[t % len(rings)].dma_start(
                    out_d[t * P:(t + 1) * P, :], ost[:])


def make_in_maps(query, key, value, Wq, bq, Wk, bk, Wv, bv, Wo, bo):
    """Shard the full inputs into the 8 per-core input dicts."""
    query, key, value, Wq, bq, Wk, bk, Wv, bv, Wo, bo = [
        np.asarray(a, dtype=np.float32)
        for a in (query, key, value, Wq, bq, Wk, bk, Wv, bv, Wo, bo)]

    def bf(a):
        return np.ascontiguousarray(a).astype(BF16_NP)

    in_maps = []
    for c in range(N_CORES):
        b, g = divmod(c, 4)
        sl = slice(g * OUTL, (g + 1) * OUTL)
        in_maps.append({
            "xq": bf(query[b].T),
            "xk": bf(key[b].T),
            "xv": bf(value[b].T),
            "wq": bf(Wq[sl, :].T),
            "wk": bf(Wk[sl, :].T),
            "wv": bf(Wv[sl, :].T),
            "wo": bf(Wo[:, sl].T),
            "bq": np.ascontiguousarray(bq[sl].reshape(OUTL, 1)),
            "bk": np.ascontiguousarray(bk[sl].reshape(OUTL, 1)),
            "bv_rep": np.ascontiguousarray(
                np.broadcast_to(bv[sl], (P, OUTL))),
        })
    return in_maps


_NC_CACHE = None


def _get_nc():
    global _NC_CACHE
    if _NC_CACHE is None:
        _NC_CACHE = build_program()
    return _NC_CACHE


def gather_out(results, bo):
    out = np.empty((2, T, D), dtype=np.float32)
    bo = np.asarray(bo, dtype=np.float32)
    for b in range(2):
        acc = results[4 * b]["out"].astype(np.float32)
        for g in range(1, 4):
            acc = acc + results[4 * b + g]["out"].astype(np.float32)
        out[b] = acc + bo
    return out


def kernel(query, key, value, Wq, bq, Wk, bk, Wv, bv, Wo, bo):
    nc = _get_nc()
    in_maps = make_in_maps(query, key, value, Wq, bq, Wk, bk, Wv, bv, Wo, bo)
    res = run_bass_kernel_spmd(nc, in_maps, list(range(N_CORES)))
    return gather_out(res.results, bo)
